# revision 51
# baseline (speedup 1.0000x reference)
"""Trainium2 Bass kernel for nn_Block_36575941492917 (ViG / gnn_message_passing).

Data-parallel over batch: 16 images -> 8 cores x 2 images.

Per-image pipeline (activations c-major (C, N) in SBUF):
  1. conv1x1 C->C + folded BN          (PE matmul + ACT bias copyback,
     Hx stored f32r; Hxb bf16 copy for the gc conv / msg subtract)
  2. 2x2 avg pool -> Y4 (= 4*Y)        (DVE strided adds)
  3. batched norms in phase A: per-tile PE transpose + ACT square-accum,
     then ONE Sqrt (+ reciprocal for Y) -> Yq = Y/||Y|| f32r and
     Rn = per-node ||hx|| permuted into sigma order via a host-provided
     112x112 permutation matmul (keeps Sqrt out of the steady loop so
     the ACT function table stays on the gelu set)
  4. TAB6: d=6 packed bf16 gather table (128 partitions = 8 groups of 16,
     groups 0-3 img0 replicas, 4-7 img1; partition pp holds ch e*16+pp)
  5. per 112-node tile, nodes in sigma order sigma(p) = (p%7)*16 + p//7
     (rel pre-permuted on host): scores s*r/2 = <x,yq> + (-rel/2)*r
     computed ENTIRELY in PSUM: f32r scores matmuls plus a bf16
     diag(r) @ rel matmul accumulated into the same bank (r = ||x col||;
     positive per-row scale keeps the ordering)
  6. top-k via DVE max8/max_index straight from PSUM (KSEL=8 drops the
     9th neighbor; measured rel err 1.77e-2 < 2e-2 gate) -> u16
     (112,KSEL) contiguous DRAM write; sigma makes this the 16-wrapped
     index layout.  Groups are software-pipelined: group j+1's scores/
     topk are emitted before group j's gather so the DVE isn't starved
     while the Pool engine runs the gather
  7. per group-of-8-tiles: 2 contiguous index loads, ONE gpsimd.ap_gather
     (d=6, 112*KSEL idxs) = KSEL neighbors x 96 ch x 8 tiles; DVE bf16
     2x tree-max over k; PE mini-transposes -> c-major;
     msg = max_k(y_j) - hxb into persistent SBUF Msg (all bf16, 2x DVE)
  8. gc conv (2C->2C bf16) + BN+GELU; fc2 + BN with the x residual added
     in-PSUM via an identity matmul -> score_map (f32r)
  9. FFN (f32r fc1 C->4C, GELU, bf16 4C->C) with the score_map residual
     added in-PSUM; BNs folded on host

Only the big ap_gather runs on gpsimd: small per-tile Pool ops cost
~1.3us launch overhead on HW (vs 95ns in the cost model) and were a
net loss every time they were tried.
"""

import numpy as np

import concourse.bass as bass
import concourse.tile as tile
from concourse import bacc, mybir
from concourse.bass_utils import run_bass_kernel_spmd
from concourse.masks import make_identity

F32 = mybir.dt.float32
F32R = mybir.dt.float32r
BF16 = mybir.dt.bfloat16
U32 = mybir.dt.uint32
U16 = mybir.dt.uint16
I16 = mybir.dt.int16
AF = mybir.ActivationFunctionType
OP = mybir.AluOpType
AX = mybir.AxisListType

B, C, H, W = 16, 96, 56, 56
N = H * W            # 3136
NR = N // 4          # 784
NCORES = 8
IPC = B // NCORES    # 2 images per core
NT = 112             # n-tile rows for the knn/topk phase
NTILES = N // NT     # 28
CHK = 448            # n-chunk for conv phases
NCHK = N // CHK      # 7
C2 = 2 * C           # 192
C4 = 4 * C           # 384
EPS = 1e-5
NEG = -1.0e30
D6 = 6               # channels packed per gathered element
PPT = C // D6        # 16 partitions per image-tile in the gather call
TPC = 4              # tiles per call per image
CALLS = NTILES // TPC  # 7 gather calls per body

# ---- build-time config -----------------------------------------------------
KSEL = 9             # neighbors gathered (9 exact, 8 = drop-the-9th approx)
F32R_SCORES = True   # f32r for conv1 + scores matmuls (4x PE speedup)

NK = NT * KSEL       # indices per tile
WPP = NT * KSEL // 16  # u16 idx words per partition in wrapped layout


def _build_nc(reps: int = 1, ndev: int = NCORES):
    nc = bacc.Bacc("TRN2", target_bir_lowering=False, debug=False,
                   num_devices=ndev)

    # ---- DRAM I/O ----
    xs = nc.dram_tensor("xs", [IPC, C, N], F32, kind="ExternalInput")
    # relp = -rel/2, sigma-permuted, bf16
    relp = nc.dram_tensor("relp", [NTILES, NT, NR], BF16, kind="ExternalInput")
    w1t = nc.dram_tensor("w1t", [C, C], F32, kind="ExternalInput")
    b1 = nc.dram_tensor("b1", [C, 1], F32, kind="ExternalInput")
    w2ta = nc.dram_tensor("w2ta", [C, C2], BF16, kind="ExternalInput")
    w2tb = nc.dram_tensor("w2tb", [C, C2], BF16, kind="ExternalInput")
    b2 = nc.dram_tensor("b2", [C, 2], F32, kind="ExternalInput")
    w3t = nc.dram_tensor("w3t", [C, 2 * C], BF16, kind="ExternalInput")
    b3 = nc.dram_tensor("b3", [C, 1], F32, kind="ExternalInput")
    w4t = nc.dram_tensor("w4t", [C, C4], BF16, kind="ExternalInput")
    b4 = nc.dram_tensor("b4", [128, 3], F32, kind="ExternalInput")
    w5t = nc.dram_tensor("w5t", [128, 3 * C], BF16, kind="ExternalInput")
    b5 = nc.dram_tensor("b5", [C, 1], F32, kind="ExternalInput")
    out_d = nc.dram_tensor("out", [IPC, C, N], F32, kind="ExternalOutput")
    # idxw[i, nt] flat (p*K+k) == wrapped layout (pw*WPP + jj*K + k)
    idxw = nc.dram_tensor("idxw", [IPC, NTILES, NT * KSEL], U16)
    psig = nc.dram_tensor("psig", [NT, NT], BF16, kind="ExternalInput")

    with tile.TileContext(nc) as tc:
        _emit(nc, tc, reps, xs, relp, w1t, b1, w2ta, w2tb, b2, w3t, b3,
              w4t, b4, w5t, b5, out_d, idxw, psig)
    nc.compile()
    return nc


def _mm_dt(ap):
    return ap.bitcast(F32R) if F32R_SCORES else ap


def _emit(nc, tc, reps, xs, relp, w1t, b1, w2ta, w2tb, b2, w3t, b3,
          w4t, b4, w5t, b5, out_d, idxw, psig):
    from contextlib import ExitStack
    ctx = ExitStack()
    with ctx:
        singles = ctx.enter_context(tc.tile_pool(name="singles", bufs=1))
        resid = ctx.enter_context(tc.tile_pool(name="resid", bufs=1))

        # identities for PE transposes
        id_f32 = singles.tile([128, 128], F32, tag="id_f32")
        make_identity(nc, id_f32)
        id_bf16 = singles.tile([128, 128], BF16, tag="id_bf16")
        make_identity(nc, id_bf16)
        id_f32r = singles.tile([128, 128], F32R, tag="id_f32r")
        nc.vector.tensor_copy(id_f32r[:], id_f32[:])
        # partition-selection matrices for the TAB6 build:
        # Psel[c, e, p] = 1 iff c == e*16 + p%16
        Psel = singles.tile([C, D6, 128], BF16, tag="Psel")
        for e in range(D6):
            for g in range(8):
                nc.vector.tensor_copy(
                    Psel[:, e, g * PPT:(g + 1) * PPT],
                    id_bf16[0:C, e * PPT:(e + 1) * PPT])

        # weights -> SBUF
        def load(name, dram, shape, dt=F32):
            t = singles.tile(shape, dt, tag=name)
            nc.sync.dma_start(out=t[:], in_=dram[:])
            return t

        w1t_s = load("w1t", w1t, [C, C])
        psig_s = load("psig", psig, [NT, NT], BF16)
        b1_s = load("b1", b1, [C, 1])
        w2ta_s = load("w2ta", w2ta, [C, C2], BF16)
        w2tb_s = load("w2tb", w2tb, [C, C2], BF16)
        b2_s = load("b2", b2, [C, 2])
        w3t_s = load("w3t", w3t, [C, 2 * C], BF16)
        b3_s = load("b3", b3, [C, 1])
        w4t_s = load("w4t", w4t, [C, C4], BF16)
        w4r_s = singles.tile([C, C4], F32R, tag="w4r")
        nc.vector.tensor_copy(w4r_s[:], w4t_s[:])
        b4_s = load("b4", b4, [128, 3])
        w5t_s = load("w5t", w5t, [128, 3 * C], BF16)
        b5_s = load("b5", b5, [C, 1])

        # full input resident in SBUF (rep-invariant, loaded once)
        Xs = [singles.tile([C, N], F32, tag=f"Xs{i}", name=f"Xs{i}")
              for i in range(IPC)]
        for i in range(IPC):
            nc.sync.dma_start(out=Xs[i][:], in_=xs[i, :, :])

        # persistent per-image activations (c-major)
        Hx = [resid.tile([C, N], F32R, tag=f"Hx{i}", name=f"Hx{i}")
              for i in range(IPC)]
        Rn = [resid.tile([NT, NTILES], F32, tag=f"Rn{i}", name=f"Rn{i}")
              for i in range(IPC)]
        Hxb = [resid.tile([C, N], BF16, tag=f"Hxb{i}", name=f"Hxb{i}")
               for i in range(IPC)]
        Smap = [resid.tile([C, N], F32R, tag=f"S{i}", name=f"S{i}")
                for i in range(IPC)]
        Yq = [resid.tile([C, NR], BF16, tag=f"Yq{i}", name=f"Yq{i}")
              for i in range(IPC)]
        Hxp = [resid.tile([C, N], BF16, tag=f"Hxp{i}", name=f"Hxp{i}")
               for i in range(IPC)]
        TAB6 = resid.tile([128, NR, D6], BF16, tag="TAB6", name="TAB6")

        def body(_iv=None):
            # single scope: PSUM rings shared across phases so rep r+1's
            # phase A overlaps rep r's E/F tail
            with (
                tc.tile_pool(name="ptmp1", bufs=1) as ptmp1,
                tc.tile_pool(name="ptmp3", bufs=2) as ptmp3,
                tc.tile_pool(name="ptmp2", bufs=3) as ptmp2,
                tc.tile_pool(name="relp_p", bufs=10) as relpool,
                tc.tile_pool(name="sp", bufs=3) as sp,
                tc.tile_pool(name="ip", bufs=12) as ip,
                tc.tile_pool(name="gp", bufs=2) as gp,
                tc.tile_pool(name="wp", bufs=3) as wp,
                tc.tile_pool(name="mp", bufs=2) as mp,
                tc.tile_pool(name="ctmp", bufs=2) as ctmp,
                tc.tile_pool(name="psS", bufs=2, space="PSUM") as psS,
                tc.tile_pool(name="psM", bufs=1, space="PSUM") as psM,
                tc.tile_pool(name="psF4", bufs=2, space="PSUM") as psF4,
            ):
                # ---------------- phase A: conv1, pool, normalize, tables -
                for i in range(IPC):
                    rssq = ptmp3.tile([NT, NTILES], F32, tag="rssq")
                    # conv1 + BN fold (X streamed per chunk); x-node norm
                    # transposes+squares interleave per finished chunk
                    for ch in range(NCHK):
                        sl = bass.ts(ch, CHK)
                        ps = psF4.tile([128, CHK], F32, tag="pf4")
                        nc.tensor.matmul(ps[:C, :], lhsT=w1t_s[:],
                                         rhs=Xs[i][:, sl],
                                         start=True, stop=True)
                        nc.scalar.activation(Hx[i][:, sl], ps[:C, :],
                                             AF.Identity,
                                             bias=b1_s[:, 0:1], scale=1.0)
                        nc.scalar.activation(Hxb[i][:, sl], ps[:C, :],
                                             AF.Identity,
                                             bias=b1_s[:, 0:1], scale=1.0)
                        hxpv = Hxp[i][:, sl].rearrange(
                            "c (t a b) -> c t a b", t=4, a=PPT, b=CALLS)
                        psv = ps[:C, :].rearrange(
                            "c (t b a) -> c t a b", t=4, b=CALLS, a=PPT)
                        nc.scalar.activation(hxpv[:], psv, AF.Identity,
                                             bias=b1_s[:, 0:1], scale=1.0)
                        for nt in range(4 * ch, 4 * ch + 4):
                            nsl = bass.ts(nt, NT)
                            pht = psS.tile([NT, C], F32R, tag="s")
                            nc.tensor.transpose(pht[:], Hx[i][:, nsl],
                                                id_f32r[:C, :C])
                            hsq = ptmp2.tile([NT, C], F32, tag="sq")
                            nc.scalar.activation(hsq[:], pht[:], AF.Square,
                                                 accum_out=rssq[:, nt:nt + 1])
                    rnr = ptmp2.tile([NT, NTILES], BF16, tag="rnr")
                    nc.scalar.activation(rnr[:], rssq[:], AF.Sqrt)
                    # permute rows into sigma order: Rn[p] = rnr[sigma(p)]
                    prn = psS.tile([NT, NTILES], F32, tag="s")
                    nc.tensor.matmul(prn[:], lhsT=psig_s[:], rhs=rnr[:],
                                     start=True, stop=True)
                    nc.scalar.activation(Rn[i][:], prn[:], AF.Copy,
                                         bias=0.0, scale=1.0)
                    # 2x2 avg pool (x4)
                    t1 = ptmp1.tile([C, N // 2], F32, tag="t1")
                    hv = Hx[i].rearrange("p (x two) -> p x two", two=2)
                    nc.vector.tensor_tensor(t1[:], hv[:, :, 0], hv[:, :, 1],
                                            op=OP.add)
                    y4 = ptmp3.tile([C, NR], F32R, tag="y4")
                    tv = t1.rearrange("p (h two w) -> p h two w", two=2, w=28)
                    nc.vector.tensor_tensor(y4[:], tv[:, :, 0, :], tv[:, :, 1, :],
                                            op=OP.add)
                    # per-m-column norms: transpose+square-accum per tile,
                    # then ONE batched Sqrt + reciprocal (avoids per-tile
                    # DVE reciprocal head-of-line stalls)
                    YMT = NR // NT  # 7
                    ptS = ptmp3.tile([NT, YMT, C], F32, tag="ptS")
                    yssq = ptmp2.tile([NT, YMT], F32, tag="yssq")
                    for mt in range(YMT):
                        msl = bass.ts(mt, NT)
                        pt = psS.tile([NT, C], F32R, tag="s")
                        nc.tensor.transpose(pt[:], y4[:, msl],
                                            id_f32r[:C, :C])
                        sq = ptmp2.tile([NT, C], F32, tag="sq")
                        nc.scalar.activation(sq[:], pt[:], AF.Square,
                                             accum_out=yssq[:, mt:mt + 1])
                        nc.scalar.activation(ptS[:, mt, :], pt[:], AF.Copy,
                                             bias=0.0, scale=1.0)
                    yrt = ptmp2.tile([NT, YMT], F32, tag="yrt")
                    nc.scalar.activation(yrt[:], yssq[:], AF.Sqrt)
                    yrq = ptmp2.tile([NT, YMT], F32, tag="yrq")
                    nc.vector.reciprocal(yrq[:], yrt[:])
                    for mt in range(YMT):
                        msl = bass.ts(mt, NT)
                        ynt = ptmp2.tile([NT, C], F32, tag="ynt")
                        nc.scalar.activation(ynt[:], ptS[:, mt, :], AF.Copy,
                                             bias=0.0, scale=yrq[:, mt:mt + 1])
                        pb = psF4.tile([128, CHK], F32, tag="pf4")
                        nc.tensor.transpose(pb[:C, :NT], ynt[:],
                                            id_f32[:NT, :NT])
                        nc.scalar.activation(Yq[i][:, msl], pb[:C, :NT],
                                             AF.Copy, bias=0.0, scale=1.0)
                    # TAB6 build on PE: partition pp of each 16-group holds
                    # channels e*16+pp of Y (= y4 * 0.25), bf16
                    y4b = ptmp3.tile([C, NR], BF16, tag="y4b")
                    nc.scalar.activation(y4b[:], y4[:], AF.Copy, bias=0.0,
                                         scale=0.25)
                    for e in range(D6):
                        pt6 = psS.tile([128, NR], F32, tag="s")
                        hsl = slice(i * 64, (i + 1) * 64)
                        nc.tensor.matmul(
                            pt6[hsl, 0:512], lhsT=Psel[:, e, hsl],
                            rhs=y4b[:, 0:512], start=True, stop=True,
                            tile_position=(0, i * 64))
                        nc.tensor.matmul(
                            pt6[hsl, 512:NR], lhsT=Psel[:, e, hsl],
                            rhs=y4b[:, 512:NR], start=True, stop=True,
                            tile_position=(0, i * 64))
                        nc.vector.tensor_copy(
                            TAB6[hsl, :, e], pt6[hsl, :])

                # -------- phase E+F interleaved: per gather-call j, the
                # conv chunk j of both images runs on PE/ACT while call
                # j+1's topk/gather occupies DVE/Pool
                def emitF(i, ch, mr):
                    sl = bass.ts(ch, CHK)
                    mrf = mr.rearrange("c a b -> c (a b)")
                    # gc conv: out 192 ch in two groups of 96
                    g1 = ctmp.tile([C, 2, CHK], BF16, tag="g1")
                    for gi in range(2):
                        gsl = bass.ts(gi, C)
                        pg = psF4.tile([128, CHK], F32, tag="pf4")
                        nc.tensor.matmul(pg[:C, :], lhsT=w2ta_s[:, gsl],
                                         rhs=Hxb[i][:, sl],
                                         start=True, stop=False)
                        nc.tensor.matmul(pg[:C, :], lhsT=w2tb_s[:, gsl],
                                         rhs=mrf,
                                         start=False, stop=True)
                        nc.scalar.activation(g1[:, gi, :], pg[:C, :], AF.Gelu,
                                             bias=b2_s[:, gi:gi + 1])
                    # fc2 + residual -> score map
                    pf = psF4.tile([128, CHK], F32, tag="pf4")
                    nc.tensor.matmul(pf[:C, :], lhsT=w3t_s[:, 0:C],
                                     rhs=g1[:, 0, :], start=True, stop=False)
                    nc.tensor.matmul(pf[:C, :], lhsT=w3t_s[:, C:2 * C],
                                     rhs=g1[:, 1, :], start=False, stop=False)
                    # residual x added in-PSUM via identity matmul
                    nc.tensor.matmul(pf[:C, :], lhsT=id_f32[:C, :C],
                                     rhs=Xs[i][:, sl], start=False, stop=True)
                    nc.scalar.activation(Smap[i][:, sl], pf[:C, :],
                                         AF.Identity, bias=b3_s[:, 0:1])
                    # FFN (fc1 f32r on Smap directly)
                    u = ctmp.tile([128, 3, CHK], BF16, tag="u")
                    for gi in range(3):
                        pu = psF4.tile([128, CHK], F32, tag="pf4")
                        nc.tensor.matmul(pu[:], lhsT=w4r_s[:, bass.ts(gi, 128)],
                                         rhs=Smap[i][:, sl],
                                         start=True, stop=True)
                        nc.scalar.activation(u[:, gi, :], pu[:], AF.Gelu,
                                             bias=b4_s[:, gi:gi + 1])
                    pv = psF4.tile([128, CHK], F32, tag="pf4")
                    for gi in range(3):
                        nc.tensor.matmul(pv[:C, :], lhsT=w5t_s[:, bass.ts(gi, C)],
                                         rhs=u[:, gi, :],
                                         start=(gi == 0), stop=False)
                    # residual smap added in-PSUM (f32r identity matmul)
                    nc.tensor.matmul(pv[:C, :], lhsT=id_f32r[:C, :C],
                                     rhs=Smap[i][:, sl],
                                     start=False, stop=True)
                    ot = ctmp.tile([C, CHK], F32, tag="ot")
                    nc.scalar.activation(ot[:], pv[:C, :], AF.Identity,
                                         bias=b5_s[:, 0:1])
                    nc.sync.dma_start(out=out_d[i, :, sl], in_=ot[:])

                def topk_tiles(j):
                    for tg in range(TPC):
                        nt = j * TPC + tg
                        nsl = bass.ts(nt, NT)
                        rel_t = relpool.tile([NT, NR], BF16, tag="rel")
                        nc.sync.dma_start(out=rel_t[:], in_=relp[nt, :, :])
                        for i in range(IPC):
                            # sigma-permuted bf16 lhsT, prebuilt in conv1
                            hxv = Hxp[i][:, nsl]
                            # scores*r/2 = <x, yq> + (-rel/2)*r,
                            # the rel*r term added INSIDE PSUM via a
                            # diag(r) @ rel bf16 matmul (PE), so the DVE
                            # only runs the max8/max_index scans
                            diag = ip.tile([NT, NT], BF16, tag="diag")
                            nc.vector.tensor_tensor(
                                diag[:], id_bf16[:NT, :NT],
                                Rn[i][:, nt:nt + 1].to_broadcast([NT, NT]),
                                op=OP.mult)
                            ps = psS.tile([128, NR], F32, tag="s")
                            nc.tensor.matmul(ps[:NT, 0:512],
                                             lhsT=hxv,
                                             rhs=Yq[i][:, 0:512],
                                             start=True, stop=False)
                            nc.tensor.matmul(ps[:NT, 0:512], lhsT=diag[:],
                                             rhs=rel_t[:, 0:512],
                                             start=False, stop=True)
                            nc.tensor.matmul(ps[:NT, 512:NR],
                                             lhsT=hxv,
                                             rhs=Yq[i][:, 512:NR],
                                             start=True, stop=False)
                            nc.tensor.matmul(ps[:NT, 512:NR], lhsT=diag[:],
                                             rhs=rel_t[:, 512:NR],
                                             start=False, stop=True)
                            s = ps[:NT, :]
                            # top-k -> ifull cols 0:KSEL u16
                            ifull = ip.tile([NT, 16], U16, tag="ifull")
                            m8 = ip.tile([NT, 8], F32, tag="m8")
                            nc.vector.max(m8[:], s)
                            nc.vector.max_index(ifull[:, 0:8], m8[:], s)
                            if KSEL == 9:
                                srep = sp.tile([NT, NR], F32, tag="srep")
                                nc.vector.match_replace(
                                    srep[:], in_to_replace=m8[:],
                                    in_values=s, imm_value=NEG)
                                m8b = ip.tile([NT, 8], F32, tag="m8b")
                                nc.vector.max(m8b[:], srep[:])
                                nc.vector.max_index(
                                    ifull[:, 8:16],
                                    m8b[:, 0:1].to_broadcast([NT, 8]), s)
                            nc.sync.dma_start(
                                out=idxw[i, nt, :].rearrange(
                                    "(p k) -> p k", k=KSEL),
                                in_=ifull[:, 0:KSEL])

                topk_tiles(0)
                topk_tiles(1)
                for j in range(CALLS):
                    # software pipeline: groups j+1/j+2's scores/topk are
                    # queued ahead of group j's gather so DVE isn't starved
                    # while the Pool engine runs the (long) gather
                    if j + 2 < CALLS:
                        topk_tiles(j + 2)
                    # wrapped index load: one contiguous DMA per image
                    w = wp.tile([128, WPP], U16, tag="w")
                    for i in range(IPC):
                        src = idxw[i, j * TPC:(j + 1) * TPC, :].rearrange(
                            "tg (pw c) -> (tg pw) c", c=WPP)
                        nc.sync.dma_start(out=w[i * 64:(i + 1) * 64, :],
                                          in_=src)
                    # ONE gather for 8 image-tiles
                    g6 = gp.tile([128, NK, D6], BF16, tag="g6")
                    nc.gpsimd.ap_gather(
                        g6[:], TAB6[:], w[:].bitcast(I16),
                        channels=128, num_elems=NR, d=D6, num_idxs=NK)
                    # bf16 tree-max over k on contiguous 96-elem slices
                    # g6 free = (jj:NT//PPT, k:KSEL, (pw e):96)
                    gk = g6.rearrange("p (jj k m) e -> p jj k (m e)",
                                      jj=CALLS, k=KSEL)
                    mx = mp.tile([128, NT * D6], BF16, tag="mx")
                    mxv = mx.rearrange("p (jj m) -> p jj m", jj=CALLS)
                    nc.vector.tensor_tensor(mxv[:], gk[:, :, 0, :],
                                            gk[:, :, 1, :], op=OP.max)
                    for kk in range(2, KSEL):
                        nc.vector.tensor_tensor(mxv[:], mxv[:], gk[:, :, kk, :],
                                                op=OP.max)
                    # per 32-partition pair (2 image-tiles): 6 transposes
                    # into one PSUM tile, then per tile ACT+transpose -> Msg
                    msgr = [mp.tile([C, TPC, NT], BF16, tag=f"msgr{i}",
                                    name=f"msgr{i}") for i in range(IPC)]
                    mxv = mx.rearrange("p (n e) -> p n e", e=D6)
                    # 6 full-width transposes (one per packed channel e)
                    # instead of 24 quadrant transposes
                    pqF = psM.tile([NT, D6, 128], BF16, tag="pq")
                    for e in range(D6):
                        nc.tensor.transpose(pqF[:, e, :], mxv[:, :, e],
                                            id_bf16[:, :])
                    for q2 in range(TPC):
                        # adjacent tile-pairs share an image: fuse the ACT
                        # de-interleave and the DVE subtract across pairs
                        i, tg0 = q2 // 2, 2 * (q2 % 2)
                        nt0 = j * TPC + tg0
                        p0 = 32 * q2
                        msgT2 = mp.tile([NT, 2, C], BF16, tag="msgT")
                        mtv = msgT2.rearrange("p t (e pp) -> p t e pp", e=D6)
                        src2 = pqF[:, :, p0:p0 + 32].rearrange(
                            "p e (t pp) -> p t e pp", t=2)
                        nc.scalar.activation(mtv[:], src2,
                                             AF.Copy, bias=0.0, scale=1.0)
                        pmt2 = psM.tile([C, 2, NT], BF16, tag="pmt2")
                        for t in range(2):
                            nc.tensor.transpose(pmt2[:, t, :], msgT2[:, t, :],
                                                id_bf16[:NT, :NT])
                        hb2 = Hxb[i][:, nt0 * NT:(nt0 + 2) * NT].rearrange(
                            "c (t n) -> c t n", t=2)
                        nc.vector.tensor_tensor(
                            msgr[i][:, tg0:tg0 + 2, :], pmt2[:],
                            hb2, op=OP.subtract)
                    # conv/FFN chunk j of both images
                    for i in range(IPC):
                        emitF(i, j, msgr[i][:])

        if reps == 1:
            body()
        else:
            with tc.For_i(0, reps, 1) as iv:
                body(iv)


# ------------------------- host side ---------------------------------------

def _fold_bn(g, b, m, v):
    inv = g / np.sqrt(v + EPS)
    return inv, b - m * inv


def _prep_weights(inp):
    f32 = np.float32
    o = {}
    inv1, sh1 = _fold_bn(inp["g_bn1_g"], inp["g_bn1_b"], inp["g_bn1_m"],
                         inp["g_bn1_v"])
    w1 = inp["g_fc1_w"] * inv1[:, None]
    b1 = inp["g_fc1_b"] * inv1 + sh1
    o["w1t"] = np.ascontiguousarray(w1.T, f32)
    o["b1"] = np.ascontiguousarray(b1[:, None], f32)

    inv2, sh2 = _fold_bn(inp["gc_bn_g"], inp["gc_bn_b"], inp["gc_bn_m"],
                         inp["gc_bn_v"])
    w2 = inp["gc_w"] * inv2[:, None]
    b2v = inp["gc_b"] * inv2 + sh2
    perm = np.concatenate([np.arange(0, C2, 2), np.arange(1, C2, 2)])
    w2p = w2[:, perm]          # stacked [hx; msg] input order
    w2T = w2p.T                # (192 in, 192 out)
    import ml_dtypes
    bf16 = ml_dtypes.bfloat16
    o["w2ta"] = np.ascontiguousarray(w2T[:C, :]).astype(bf16)
    o["w2tb"] = np.ascontiguousarray(w2T[C:, :]).astype(bf16)
    o["b2"] = np.ascontiguousarray(
        np.stack([b2v[:C], b2v[C:]], axis=1), f32)

    inv3, sh3 = _fold_bn(inp["g_bn2_g"], inp["g_bn2_b"], inp["g_bn2_m"],
                         inp["g_bn2_v"])
    w3 = inp["g_fc2_w"] * inv3[:, None]    # (96, 192)
    b3v = inp["g_fc2_b"] * inv3 + sh3
    w3T = w3.T                              # (192, 96)
    o["w3t"] = np.ascontiguousarray(
        np.concatenate([w3T[:C, :], w3T[C:, :]], axis=1)).astype(bf16)
    o["b3"] = np.ascontiguousarray(b3v[:, None], f32)

    inv4, sh4 = _fold_bn(inp["f_bn1_g"], inp["f_bn1_b"], inp["f_bn1_m"],
                         inp["f_bn1_v"])
    w4 = inp["f_fc1_w"] * inv4[:, None]    # (384, 96)
    b4v = inp["f_fc1_b"] * inv4 + sh4
    o["w4t"] = np.ascontiguousarray(w4.T).astype(bf16)   # (96, 384)
    o["b4"] = np.ascontiguousarray(b4v.reshape(3, 128).T, f32)  # (128, 3)

    inv5, sh5 = _fold_bn(inp["f_bn2_g"], inp["f_bn2_b"], inp["f_bn2_m"],
                         inp["f_bn2_v"])
    w5 = inp["f_fc2_w"] * inv5[:, None]    # (96, 384)
    b5v = inp["f_fc2_b"] * inv5 + sh5
    w5T = w5.T                              # (384, 96)
    o["w5t"] = np.ascontiguousarray(
        np.concatenate([w5T[gi * 128:(gi + 1) * 128, :] for gi in range(3)],
                       axis=1)).astype(bf16)  # (128, 288)
    o["b5"] = np.ascontiguousarray(b5v[:, None], f32)
    return o


_NC_CACHE = {}

# sigma node permutation within each 112-tile: partition p <- node sigma(p)
_SIGMA = np.array([(p % 7) * 16 + p // 7 for p in range(NT)])


def get_nc(reps: int = 1, ndev: int = NCORES):
    key = (reps, ndev)
    if key not in _NC_CACHE:
        _NC_CACHE[key] = _build_nc(reps, ndev)
    return _NC_CACHE[key]


def make_in_maps(inputs, ncores: int = NCORES):
    import ml_dtypes
    wts = _prep_weights({k: np.asarray(v) for k, v in inputs.items()})
    x = np.asarray(inputs["x"], np.float32).reshape(B, C, N)
    relf = np.asarray(inputs["rel_pos"], np.float32).reshape(N, NR)
    relperm = np.ascontiguousarray(
        (-0.5 * relf).reshape(NTILES, NT, NR)[:, _SIGMA, :]).astype(
            ml_dtypes.bfloat16)
    psig_h = np.zeros((NT, NT), np.float32)
    psig_h[_SIGMA, np.arange(NT)] = 1.0
    psig_h = psig_h.astype(ml_dtypes.bfloat16)
    in_maps = []
    for c in range(ncores):
        m = {"xs": np.ascontiguousarray(x[c * IPC:(c + 1) * IPC]),
             "relp": relperm, "psig": psig_h}
        m.update(wts)
        in_maps.append(m)
    return in_maps


def run(inputs, reps: int = 1):
    nc = get_nc(reps)
    in_maps = make_in_maps(inputs)
    res = run_bass_kernel_spmd(nc, in_maps, list(range(NCORES)))
    out = np.concatenate([res.results[c]["out"] for c in range(NCORES)],
                         axis=0)
    return out.reshape(B, C, H, W)


def kernel(**inputs) -> np.ndarray:
    return run(inputs, reps=1)


# revision 52
# speedup vs baseline: 1.0486x; 1.0486x over previous
"""Trainium2 Bass kernel for nn_Block_36575941492917 (ViG / gnn_message_passing).

Data-parallel over batch: 16 images -> 8 cores x 2 images.

Per-image pipeline (activations c-major (C, N) in SBUF):
  1. conv1x1 C->C + folded BN          (PE matmul + ACT bias copyback,
     Hx stored f32r; Hxb bf16 copy for the gc conv / msg subtract)
  2. 2x2 avg pool -> Y4 (= 4*Y)        (DVE strided adds)
  3. batched norms in phase A: per-tile PE transpose + ACT square-accum,
     then ONE Sqrt (+ reciprocal for Y) -> Yq = Y/||Y|| f32r and
     Rn = per-node ||hx|| permuted into sigma order via a host-provided
     112x112 permutation matmul (keeps Sqrt out of the steady loop so
     the ACT function table stays on the gelu set)
  4. TAB6: d=6 packed bf16 gather table (128 partitions = 8 groups of 16,
     groups 0-3 img0 replicas, 4-7 img1; partition pp holds ch e*16+pp)
  5. per 112-node tile, nodes in sigma order sigma(p) = (p%7)*16 + p//7
     (rel pre-permuted on host): scores s*r/2 = <x,yq> + (-rel/2)*r
     computed ENTIRELY in PSUM: f32r scores matmuls plus a bf16
     diag(r) @ rel matmul accumulated into the same bank (r = ||x col||;
     positive per-row scale keeps the ordering)
  6. top-k via DVE max8/max_index straight from PSUM (KSEL=8 drops the
     9th neighbor; measured rel err 1.77e-2 < 2e-2 gate) -> u16
     (112,KSEL) contiguous DRAM write; sigma makes this the 16-wrapped
     index layout.  Groups are software-pipelined: group j+1's scores/
     topk are emitted before group j's gather so the DVE isn't starved
     while the Pool engine runs the gather
  7. per group-of-8-tiles: 2 contiguous index loads, ONE gpsimd.ap_gather
     (d=6, 112*KSEL idxs) = KSEL neighbors x 96 ch x 8 tiles; DVE bf16
     2x tree-max over k; PE mini-transposes -> c-major;
     msg = max_k(y_j) - hxb into persistent SBUF Msg (all bf16, 2x DVE)
  8. gc conv (2C->2C bf16) + BN+GELU; fc2 + BN with the x residual added
     in-PSUM via an identity matmul -> score_map (f32r)
  9. FFN (f32r fc1 C->4C, GELU, bf16 4C->C) with the score_map residual
     added in-PSUM; BNs folded on host

Only the big ap_gather runs on gpsimd: small per-tile Pool ops cost
~1.3us launch overhead on HW (vs 95ns in the cost model) and were a
net loss every time they were tried.
"""

import numpy as np

import concourse.bass as bass
import concourse.tile as tile
from concourse import bacc, mybir
from concourse.bass_utils import run_bass_kernel_spmd
from concourse.masks import make_identity

F32 = mybir.dt.float32
F32R = mybir.dt.float32r
BF16 = mybir.dt.bfloat16
U32 = mybir.dt.uint32
U16 = mybir.dt.uint16
I16 = mybir.dt.int16
AF = mybir.ActivationFunctionType
OP = mybir.AluOpType
AX = mybir.AxisListType

B, C, H, W = 16, 96, 56, 56
N = H * W            # 3136
NR = N // 4          # 784
NCORES = 8
IPC = B // NCORES    # 2 images per core
NT = 112             # n-tile rows for the knn/topk phase
NTILES = N // NT     # 28
CHK = 448            # n-chunk for conv phases
NCHK = N // CHK      # 7
C2 = 2 * C           # 192
C4 = 4 * C           # 384
EPS = 1e-5
NEG = -1.0e30
D6 = 6               # channels packed per gathered element
PPT = C // D6        # 16 partitions per image-tile in the gather call
TPC = 4              # tiles per call per image
CALLS = NTILES // TPC  # 7 gather calls per body

# ---- build-time config -----------------------------------------------------
KSEL = 9             # neighbors gathered (9 exact, 8 = drop-the-9th approx)
F32R_SCORES = True   # f32r for conv1 + scores matmuls (4x PE speedup)

NK = NT * KSEL       # indices per tile
WPP = NT * KSEL // 16  # u16 idx words per partition in wrapped layout


def _build_nc(reps: int = 1, ndev: int = NCORES):
    nc = bacc.Bacc("TRN2", target_bir_lowering=False, debug=False,
                   num_devices=ndev)

    # ---- DRAM I/O ----
    xs = nc.dram_tensor("xs", [IPC, C, N], F32, kind="ExternalInput")
    # relp = -rel/2, sigma-permuted, bf16
    relp = nc.dram_tensor("relp", [NTILES, NT, NR], BF16, kind="ExternalInput")
    w1t = nc.dram_tensor("w1t", [C, C], F32, kind="ExternalInput")
    b1 = nc.dram_tensor("b1", [C, 1], F32, kind="ExternalInput")
    w2ta = nc.dram_tensor("w2ta", [C, C2], BF16, kind="ExternalInput")
    w2tb = nc.dram_tensor("w2tb", [C, C2], BF16, kind="ExternalInput")
    b2 = nc.dram_tensor("b2", [C, 2], F32, kind="ExternalInput")
    w3t = nc.dram_tensor("w3t", [C, 2 * C], BF16, kind="ExternalInput")
    b3 = nc.dram_tensor("b3", [C, 1], F32, kind="ExternalInput")
    w4t = nc.dram_tensor("w4t", [C, C4], BF16, kind="ExternalInput")
    b4 = nc.dram_tensor("b4", [128, 3], F32, kind="ExternalInput")
    w5t = nc.dram_tensor("w5t", [128, 3 * C], BF16, kind="ExternalInput")
    b5 = nc.dram_tensor("b5", [C, 1], F32, kind="ExternalInput")
    out_d = nc.dram_tensor("out", [IPC, C, N], F32, kind="ExternalOutput")
    # idxw[i, nt] flat (p*K+k) == wrapped layout (pw*WPP + jj*K + k)
    idxw = nc.dram_tensor("idxw", [IPC, NTILES, NT * KSEL], U16)
    psig = nc.dram_tensor("psig", [NT, NT], BF16, kind="ExternalInput")

    with tile.TileContext(nc) as tc:
        _emit(nc, tc, reps, xs, relp, w1t, b1, w2ta, w2tb, b2, w3t, b3,
              w4t, b4, w5t, b5, out_d, idxw, psig)
    nc.compile()
    return nc


def _mm_dt(ap):
    return ap.bitcast(F32R) if F32R_SCORES else ap


def _emit(nc, tc, reps, xs, relp, w1t, b1, w2ta, w2tb, b2, w3t, b3,
          w4t, b4, w5t, b5, out_d, idxw, psig):
    from contextlib import ExitStack
    ctx = ExitStack()
    with ctx:
        singles = ctx.enter_context(tc.tile_pool(name="singles", bufs=1))
        resid = ctx.enter_context(tc.tile_pool(name="resid", bufs=1))

        # identities for PE transposes
        id_f32 = singles.tile([128, 128], F32, tag="id_f32")
        make_identity(nc, id_f32)
        id_bf16 = singles.tile([128, 128], BF16, tag="id_bf16")
        make_identity(nc, id_bf16)
        id_f32r = singles.tile([128, 128], F32R, tag="id_f32r")
        nc.vector.tensor_copy(id_f32r[:], id_f32[:])
        # partition-selection matrices for the TAB6 build:
        # Psel[c, e, p] = 1 iff c == e*16 + p%16
        Psel = singles.tile([C, D6, 128], BF16, tag="Psel")
        for e in range(D6):
            for g in range(8):
                nc.vector.tensor_copy(
                    Psel[:, e, g * PPT:(g + 1) * PPT],
                    id_bf16[0:C, e * PPT:(e + 1) * PPT])

        # weights -> SBUF
        def load(name, dram, shape, dt=F32):
            t = singles.tile(shape, dt, tag=name)
            nc.sync.dma_start(out=t[:], in_=dram[:])
            return t

        w1t_s = load("w1t", w1t, [C, C])
        psig_s = load("psig", psig, [NT, NT], BF16)
        b1_s = load("b1", b1, [C, 1])
        w2ta_s = load("w2ta", w2ta, [C, C2], BF16)
        w2tb_s = load("w2tb", w2tb, [C, C2], BF16)
        b2_s = load("b2", b2, [C, 2])
        w3t_s = load("w3t", w3t, [C, 2 * C], BF16)
        b3_s = load("b3", b3, [C, 1])
        w4t_s = load("w4t", w4t, [C, C4], BF16)
        w4r_s = singles.tile([C, C4], F32R, tag="w4r")
        nc.vector.tensor_copy(w4r_s[:], w4t_s[:])
        b4_s = load("b4", b4, [128, 3])
        w5t_s = load("w5t", w5t, [128, 3 * C], BF16)
        b5_s = load("b5", b5, [C, 1])

        # full input resident in SBUF (rep-invariant, loaded once)
        Xs = [singles.tile([C, N], F32, tag=f"Xs{i}", name=f"Xs{i}")
              for i in range(IPC)]
        for i in range(IPC):
            nc.sync.dma_start(out=Xs[i][:], in_=xs[i, :, :])

        # persistent per-image activations (c-major)
        Hx = [resid.tile([C, N], F32R, tag=f"Hx{i}", name=f"Hx{i}")
              for i in range(IPC)]
        Rn = [resid.tile([NT, NTILES], F32, tag=f"Rn{i}", name=f"Rn{i}")
              for i in range(IPC)]
        Hxb = [resid.tile([C, N], BF16, tag=f"Hxb{i}", name=f"Hxb{i}")
               for i in range(IPC)]
        Smap = [resid.tile([C, N], F32R, tag=f"S{i}", name=f"S{i}")
                for i in range(IPC)]
        Yq = [resid.tile([C, NR], BF16, tag=f"Yq{i}", name=f"Yq{i}")
              for i in range(IPC)]
        Hxp = [resid.tile([C, N], BF16, tag=f"Hxp{i}", name=f"Hxp{i}")
               for i in range(IPC)]
        TAB6 = resid.tile([128, NR, D6], BF16, tag="TAB6", name="TAB6")

        def body(_iv=None):
            # single scope: PSUM rings shared across phases so rep r+1's
            # phase A overlaps rep r's E/F tail
            with (
                tc.tile_pool(name="ptmp1", bufs=1) as ptmp1,
                tc.tile_pool(name="ptmp3", bufs=2) as ptmp3,
                tc.tile_pool(name="ptmp2", bufs=3) as ptmp2,
                tc.tile_pool(name="relp_p", bufs=10) as relpool,
                tc.tile_pool(name="sp", bufs=3) as sp,
                tc.tile_pool(name="ip", bufs=12) as ip,
                tc.tile_pool(name="gp", bufs=2) as gp,
                tc.tile_pool(name="wp", bufs=3) as wp,
                tc.tile_pool(name="mp", bufs=2) as mp,
                tc.tile_pool(name="ctmp", bufs=2) as ctmp,
                tc.tile_pool(name="psS", bufs=2, space="PSUM") as psS,
                tc.tile_pool(name="psM", bufs=1, space="PSUM") as psM,
                tc.tile_pool(name="psF4", bufs=2, space="PSUM") as psF4,
            ):
                # ---------------- phase A: conv1, pool, normalize, tables -
                for i in range(IPC):
                    rssq = ptmp3.tile([NT, NTILES], F32, tag="rssq")
                    # conv1 + BN fold (X streamed per chunk); x-node norm
                    # transposes+squares interleave per finished chunk
                    for ch in range(NCHK):
                        sl = bass.ts(ch, CHK)
                        ps = psF4.tile([128, CHK], F32, tag="pf4")
                        nc.tensor.matmul(ps[:C, :], lhsT=w1t_s[:],
                                         rhs=Xs[i][:, sl],
                                         start=True, stop=True)
                        nc.scalar.activation(Hx[i][:, sl], ps[:C, :],
                                             AF.Identity,
                                             bias=b1_s[:, 0:1], scale=1.0)
                        nc.scalar.activation(Hxb[i][:, sl], ps[:C, :],
                                             AF.Identity,
                                             bias=b1_s[:, 0:1], scale=1.0)
                        hxpv = Hxp[i][:, sl].rearrange(
                            "c (t a b) -> c t a b", t=4, a=PPT, b=CALLS)
                        psv = ps[:C, :].rearrange(
                            "c (t b a) -> c t a b", t=4, b=CALLS, a=PPT)
                        nc.scalar.activation(hxpv[:], psv, AF.Identity,
                                             bias=b1_s[:, 0:1], scale=1.0)
                        for nt in range(4 * ch, 4 * ch + 4):
                            nsl = bass.ts(nt, NT)
                            pht = psS.tile([NT, C], F32R, tag="s")
                            nc.tensor.transpose(pht[:], Hx[i][:, nsl],
                                                id_f32r[:C, :C])
                            hsq = ptmp2.tile([NT, C], F32, tag="sq")
                            nc.scalar.activation(hsq[:], pht[:], AF.Square,
                                                 accum_out=rssq[:, nt:nt + 1])
                    rnr = ptmp2.tile([NT, NTILES], BF16, tag="rnr")
                    nc.scalar.activation(rnr[:], rssq[:], AF.Sqrt)
                    # permute rows into sigma order: Rn[p] = rnr[sigma(p)]
                    prn = psS.tile([NT, NTILES], F32, tag="s")
                    nc.tensor.matmul(prn[:], lhsT=psig_s[:], rhs=rnr[:],
                                     start=True, stop=True)
                    nc.scalar.activation(Rn[i][:], prn[:], AF.Copy,
                                         bias=0.0, scale=1.0)
                    # 2x2 avg pool (x4)
                    t1 = ptmp1.tile([C, N // 2], F32, tag="t1")
                    hv = Hx[i].rearrange("p (x two) -> p x two", two=2)
                    nc.vector.tensor_tensor(t1[:], hv[:, :, 0], hv[:, :, 1],
                                            op=OP.add)
                    y4 = ptmp3.tile([C, NR], F32R, tag="y4")
                    tv = t1.rearrange("p (h two w) -> p h two w", two=2, w=28)
                    nc.vector.tensor_tensor(y4[:], tv[:, :, 0, :], tv[:, :, 1, :],
                                            op=OP.add)
                    # per-m-column norms: transpose+square-accum per tile,
                    # then ONE batched Sqrt + reciprocal (avoids per-tile
                    # DVE reciprocal head-of-line stalls)
                    YMT = NR // NT  # 7
                    ptS = ptmp3.tile([NT, YMT, C], F32, tag="ptS")
                    yssq = ptmp2.tile([NT, YMT], F32, tag="yssq")
                    for mt in range(YMT):
                        msl = bass.ts(mt, NT)
                        pt = psS.tile([NT, C], F32R, tag="s")
                        nc.tensor.transpose(pt[:], y4[:, msl],
                                            id_f32r[:C, :C])
                        sq = ptmp2.tile([NT, C], F32, tag="sq")
                        nc.scalar.activation(sq[:], pt[:], AF.Square,
                                             accum_out=yssq[:, mt:mt + 1])
                        nc.scalar.activation(ptS[:, mt, :], pt[:], AF.Copy,
                                             bias=0.0, scale=1.0)
                    yrt = ptmp2.tile([NT, YMT], F32, tag="yrt")
                    nc.scalar.activation(yrt[:], yssq[:], AF.Sqrt)
                    yrq = ptmp2.tile([NT, YMT], F32, tag="yrq")
                    nc.vector.reciprocal(yrq[:], yrt[:])
                    for mt in range(YMT):
                        msl = bass.ts(mt, NT)
                        ynt = ptmp2.tile([NT, C], F32, tag="ynt")
                        nc.scalar.activation(ynt[:], ptS[:, mt, :], AF.Copy,
                                             bias=0.0, scale=yrq[:, mt:mt + 1])
                        pb = psF4.tile([128, CHK], F32, tag="pf4")
                        nc.tensor.transpose(pb[:C, :NT], ynt[:],
                                            id_f32[:NT, :NT])
                        nc.scalar.activation(Yq[i][:, msl], pb[:C, :NT],
                                             AF.Copy, bias=0.0, scale=1.0)
                    # TAB6 build on PE: partition pp of each 16-group holds
                    # channels e*16+pp of Y (= y4 * 0.25), bf16
                    y4b = ptmp3.tile([C, NR], BF16, tag="y4b")
                    nc.scalar.activation(y4b[:], y4[:], AF.Copy, bias=0.0,
                                         scale=0.25)
                    for e in range(D6):
                        pt6 = psS.tile([128, NR], F32, tag="s")
                        hsl = slice(i * 64, (i + 1) * 64)
                        nc.tensor.matmul(
                            pt6[hsl, 0:512], lhsT=Psel[:, e, hsl],
                            rhs=y4b[:, 0:512], start=True, stop=True,
                            tile_position=(0, i * 64))
                        nc.tensor.matmul(
                            pt6[hsl, 512:NR], lhsT=Psel[:, e, hsl],
                            rhs=y4b[:, 512:NR], start=True, stop=True,
                            tile_position=(0, i * 64))
                        nc.vector.tensor_copy(
                            TAB6[hsl, :, e], pt6[hsl, :])

                # -------- phase E+F interleaved: per gather-call j, the
                # conv chunk j of both images runs on PE/ACT while call
                # j+1's topk/gather occupies DVE/Pool
                def emitF(i, ch, mr):
                    sl = bass.ts(ch, CHK)
                    mrf = mr.rearrange("c a b -> c (a b)")
                    # gc conv: out 192 ch in two groups of 96
                    g1 = ctmp.tile([C, 2, CHK], BF16, tag="g1")
                    for gi in range(2):
                        gsl = bass.ts(gi, C)
                        pg = psF4.tile([128, CHK], F32, tag="pf4")
                        nc.tensor.matmul(pg[:C, :], lhsT=w2ta_s[:, gsl],
                                         rhs=Hxb[i][:, sl],
                                         start=True, stop=False)
                        nc.tensor.matmul(pg[:C, :], lhsT=w2tb_s[:, gsl],
                                         rhs=mrf,
                                         start=False, stop=True)
                        nc.scalar.activation(g1[:, gi, :], pg[:C, :], AF.Gelu,
                                             bias=b2_s[:, gi:gi + 1])
                    # fc2 + residual -> score map
                    pf = psF4.tile([128, CHK], F32, tag="pf4")
                    nc.tensor.matmul(pf[:C, :], lhsT=w3t_s[:, 0:C],
                                     rhs=g1[:, 0, :], start=True, stop=False)
                    nc.tensor.matmul(pf[:C, :], lhsT=w3t_s[:, C:2 * C],
                                     rhs=g1[:, 1, :], start=False, stop=False)
                    # residual x added in-PSUM via identity matmul
                    nc.tensor.matmul(pf[:C, :], lhsT=id_f32[:C, :C],
                                     rhs=Xs[i][:, sl], start=False, stop=True)
                    nc.scalar.activation(Smap[i][:, sl], pf[:C, :],
                                         AF.Identity, bias=b3_s[:, 0:1])
                    # FFN (fc1 f32r on Smap directly)
                    u = ctmp.tile([128, 3, CHK], BF16, tag="u")
                    for gi in range(3):
                        pu = psF4.tile([128, CHK], F32, tag="pf4")
                        nc.tensor.matmul(pu[:], lhsT=w4r_s[:, bass.ts(gi, 128)],
                                         rhs=Smap[i][:, sl],
                                         start=True, stop=True)
                        nc.scalar.activation(u[:, gi, :], pu[:], AF.Gelu,
                                             bias=b4_s[:, gi:gi + 1])
                    pv = psF4.tile([128, CHK], F32, tag="pf4")
                    for gi in range(3):
                        nc.tensor.matmul(pv[:C, :], lhsT=w5t_s[:, bass.ts(gi, C)],
                                         rhs=u[:, gi, :],
                                         start=(gi == 0), stop=False)
                    # residual smap added in-PSUM (f32r identity matmul)
                    nc.tensor.matmul(pv[:C, :], lhsT=id_f32r[:C, :C],
                                     rhs=Smap[i][:, sl],
                                     start=False, stop=True)
                    ot = ctmp.tile([C, CHK], F32, tag="ot")
                    nc.scalar.activation(ot[:], pv[:C, :], AF.Identity,
                                         bias=b5_s[:, 0:1])
                    nc.sync.dma_start(out=out_d[i, :, sl], in_=ot[:])

                def topk_tiles(j):
                    for tg in range(TPC):
                        nt = j * TPC + tg
                        nsl = bass.ts(nt, NT)
                        rel_t = relpool.tile([NT, NR], BF16, tag="rel")
                        nc.sync.dma_start(out=rel_t[:], in_=relp[nt, :, :])
                        for i in range(IPC):
                            # sigma-permuted bf16 lhsT, prebuilt in conv1
                            hxv = Hxp[i][:, nsl]
                            # scores*r/2 = <x, yq> + (-rel/2)*r,
                            # the rel*r term added INSIDE PSUM via a
                            # diag(r) @ rel bf16 matmul (PE), so the DVE
                            # only runs the max8/max_index scans
                            diag = ip.tile([NT, NT], BF16, tag="diag")
                            nc.vector.tensor_tensor(
                                diag[:], id_bf16[:NT, :NT],
                                Rn[i][:, nt:nt + 1].to_broadcast([NT, NT]),
                                op=OP.mult)
                            ps = psS.tile([128, NR], F32, tag="s")
                            nc.tensor.matmul(ps[:NT, 0:512],
                                             lhsT=hxv,
                                             rhs=Yq[i][:, 0:512],
                                             start=True, stop=False)
                            nc.tensor.matmul(ps[:NT, 0:512], lhsT=diag[:],
                                             rhs=rel_t[:, 0:512],
                                             start=False, stop=True)
                            nc.tensor.matmul(ps[:NT, 512:NR],
                                             lhsT=hxv,
                                             rhs=Yq[i][:, 512:NR],
                                             start=True, stop=False)
                            nc.tensor.matmul(ps[:NT, 512:NR], lhsT=diag[:],
                                             rhs=rel_t[:, 512:NR],
                                             start=False, stop=True)
                            s = ps[:NT, :]
                            # top-k -> ifull cols 0:KSEL u16
                            ifull = ip.tile([NT, 16], U16, tag="ifull")
                            m8 = ip.tile([NT, 8], F32, tag="m8")
                            nc.vector.max(m8[:], s)
                            nc.vector.max_index(ifull[:, 0:8], m8[:], s)
                            if KSEL == 9:
                                srep = sp.tile([NT, NR], F32, tag="srep")
                                nc.vector.match_replace(
                                    srep[:], in_to_replace=m8[:],
                                    in_values=s, imm_value=NEG)
                                m8b = ip.tile([NT, 8], F32, tag="m8b")
                                nc.vector.max(m8b[:], srep[:])
                                nc.vector.max_index(
                                    ifull[:, 8:16],
                                    m8b[:, 0:1].to_broadcast([NT, 8]), s)
                            nc.sync.dma_start(
                                out=idxw[i, nt, :].rearrange(
                                    "(p k) -> p k", k=KSEL),
                                in_=ifull[:, 0:KSEL])

                topk_tiles(0)
                topk_tiles(1)
                for j in range(CALLS):
                    # software pipeline: groups j+1/j+2's scores/topk are
                    # queued ahead of group j's gather so DVE isn't starved
                    # while the Pool engine runs the (long) gather
                    if j + 2 < CALLS:
                        topk_tiles(j + 2)
                    # wrapped index load: one contiguous DMA per image
                    w = wp.tile([128, WPP], U16, tag="w")
                    for i in range(IPC):
                        src = idxw[i, j * TPC:(j + 1) * TPC, :].rearrange(
                            "tg (pw c) -> (tg pw) c", c=WPP)
                        nc.sync.dma_start(out=w[i * 64:(i + 1) * 64, :],
                                          in_=src)
                    # ONE gather for 8 image-tiles
                    g6 = gp.tile([128, NK, D6], BF16, tag="g6")
                    nc.gpsimd.ap_gather(
                        g6[:], TAB6[:], w[:].bitcast(I16),
                        channels=128, num_elems=NR, d=D6, num_idxs=NK)
                    # bf16 tree-max over k on contiguous 96-elem slices
                    # g6 free = (jj:NT//PPT, k:KSEL, (pw e):96)
                    gk = g6.rearrange("p (jj k m) e -> p jj k (m e)",
                                      jj=CALLS, k=KSEL)
                    mx = mp.tile([128, NT * D6], BF16, tag="mx")
                    mxv = mx.rearrange("p (jj m) -> p jj m", jj=CALLS)
                    nc.vector.tensor_tensor(mxv[:], gk[:, :, 0, :],
                                            gk[:, :, 1, :], op=OP.max)
                    for kk in range(2, KSEL):
                        nc.vector.tensor_tensor(mxv[:], mxv[:], gk[:, :, kk, :],
                                                op=OP.max)
                    # per 32-partition pair (2 image-tiles): 6 transposes
                    # into one PSUM tile, then per tile ACT+transpose -> Msg
                    msgr = [mp.tile([C, TPC, NT], BF16, tag=f"msgr{i}",
                                    name=f"msgr{i}") for i in range(IPC)]
                    mxv = mx.rearrange("p (n e) -> p n e", e=D6)
                    # 6 full-width transposes (one per packed channel e)
                    # instead of 24 quadrant transposes
                    pqF = psM.tile([NT, D6, 128], BF16, tag="pq")
                    for e in range(D6):
                        nc.tensor.transpose(pqF[:, e, :], mxv[:, :, e],
                                            id_bf16[:, :])
                    for g in range(2 * TPC):
                        i, tg = g // TPC, g % TPC
                        nt = j * TPC + tg
                        nsl = bass.ts(nt, NT)
                        p0 = 16 * g
                        msgT = mp.tile([NT, C], BF16, tag="msgT")
                        mtv = msgT.rearrange("p (e pp) -> p e pp", e=D6)
                        nc.scalar.activation(
                            mtv[:], pqF[:, :, p0:p0 + PPT],
                            AF.Copy, bias=0.0, scale=1.0)
                        pmt2 = psM.tile([C, NT], BF16, tag="pmt2")
                        nc.tensor.transpose(pmt2[:], msgT[:],
                                            id_bf16[:NT, :NT])
                        nc.vector.tensor_tensor(
                            msgr[i][:, tg, :], pmt2[:],
                            Hxb[i][:, nsl], op=OP.subtract)
                    # conv/FFN chunk j of both images
                    for i in range(IPC):
                        emitF(i, j, msgr[i][:])

        if reps == 1:
            body()
        else:
            with tc.For_i(0, reps, 1) as iv:
                body(iv)


# ------------------------- host side ---------------------------------------

def _fold_bn(g, b, m, v):
    inv = g / np.sqrt(v + EPS)
    return inv, b - m * inv


def _prep_weights(inp):
    f32 = np.float32
    o = {}
    inv1, sh1 = _fold_bn(inp["g_bn1_g"], inp["g_bn1_b"], inp["g_bn1_m"],
                         inp["g_bn1_v"])
    w1 = inp["g_fc1_w"] * inv1[:, None]
    b1 = inp["g_fc1_b"] * inv1 + sh1
    o["w1t"] = np.ascontiguousarray(w1.T, f32)
    o["b1"] = np.ascontiguousarray(b1[:, None], f32)

    inv2, sh2 = _fold_bn(inp["gc_bn_g"], inp["gc_bn_b"], inp["gc_bn_m"],
                         inp["gc_bn_v"])
    w2 = inp["gc_w"] * inv2[:, None]
    b2v = inp["gc_b"] * inv2 + sh2
    perm = np.concatenate([np.arange(0, C2, 2), np.arange(1, C2, 2)])
    w2p = w2[:, perm]          # stacked [hx; msg] input order
    w2T = w2p.T                # (192 in, 192 out)
    import ml_dtypes
    bf16 = ml_dtypes.bfloat16
    o["w2ta"] = np.ascontiguousarray(w2T[:C, :]).astype(bf16)
    o["w2tb"] = np.ascontiguousarray(w2T[C:, :]).astype(bf16)
    o["b2"] = np.ascontiguousarray(
        np.stack([b2v[:C], b2v[C:]], axis=1), f32)

    inv3, sh3 = _fold_bn(inp["g_bn2_g"], inp["g_bn2_b"], inp["g_bn2_m"],
                         inp["g_bn2_v"])
    w3 = inp["g_fc2_w"] * inv3[:, None]    # (96, 192)
    b3v = inp["g_fc2_b"] * inv3 + sh3
    w3T = w3.T                              # (192, 96)
    o["w3t"] = np.ascontiguousarray(
        np.concatenate([w3T[:C, :], w3T[C:, :]], axis=1)).astype(bf16)
    o["b3"] = np.ascontiguousarray(b3v[:, None], f32)

    inv4, sh4 = _fold_bn(inp["f_bn1_g"], inp["f_bn1_b"], inp["f_bn1_m"],
                         inp["f_bn1_v"])
    w4 = inp["f_fc1_w"] * inv4[:, None]    # (384, 96)
    b4v = inp["f_fc1_b"] * inv4 + sh4
    o["w4t"] = np.ascontiguousarray(w4.T).astype(bf16)   # (96, 384)
    o["b4"] = np.ascontiguousarray(b4v.reshape(3, 128).T, f32)  # (128, 3)

    inv5, sh5 = _fold_bn(inp["f_bn2_g"], inp["f_bn2_b"], inp["f_bn2_m"],
                         inp["f_bn2_v"])
    w5 = inp["f_fc2_w"] * inv5[:, None]    # (96, 384)
    b5v = inp["f_fc2_b"] * inv5 + sh5
    w5T = w5.T                              # (384, 96)
    o["w5t"] = np.ascontiguousarray(
        np.concatenate([w5T[gi * 128:(gi + 1) * 128, :] for gi in range(3)],
                       axis=1)).astype(bf16)  # (128, 288)
    o["b5"] = np.ascontiguousarray(b5v[:, None], f32)
    return o


_NC_CACHE = {}

# sigma node permutation within each 112-tile: partition p <- node sigma(p)
_SIGMA = np.array([(p % 7) * 16 + p // 7 for p in range(NT)])


def get_nc(reps: int = 1, ndev: int = NCORES):
    key = (reps, ndev)
    if key not in _NC_CACHE:
        _NC_CACHE[key] = _build_nc(reps, ndev)
    return _NC_CACHE[key]


def make_in_maps(inputs, ncores: int = NCORES):
    import ml_dtypes
    wts = _prep_weights({k: np.asarray(v) for k, v in inputs.items()})
    x = np.asarray(inputs["x"], np.float32).reshape(B, C, N)
    relf = np.asarray(inputs["rel_pos"], np.float32).reshape(N, NR)
    relperm = np.ascontiguousarray(
        (-0.5 * relf).reshape(NTILES, NT, NR)[:, _SIGMA, :]).astype(
            ml_dtypes.bfloat16)
    psig_h = np.zeros((NT, NT), np.float32)
    psig_h[_SIGMA, np.arange(NT)] = 1.0
    psig_h = psig_h.astype(ml_dtypes.bfloat16)
    in_maps = []
    for c in range(ncores):
        m = {"xs": np.ascontiguousarray(x[c * IPC:(c + 1) * IPC]),
             "relp": relperm, "psig": psig_h}
        m.update(wts)
        in_maps.append(m)
    return in_maps


def run(inputs, reps: int = 1):
    nc = get_nc(reps)
    in_maps = make_in_maps(inputs)
    res = run_bass_kernel_spmd(nc, in_maps, list(range(NCORES)))
    out = np.concatenate([res.results[c]["out"] for c in range(NCORES)],
                         axis=0)
    return out.reshape(B, C, H, W)


def kernel(**inputs) -> np.ndarray:
    return run(inputs, reps=1)
